# revision 4
# baseline (speedup 1.0000x reference)
"""Grouped MLP (MoE) Trainium2 kernel: 8 experts x 1024 tokens, H=2048, I=5632, GLU.

Expert-parallel sharding: core i handles expert i's full MLP (zero cross-core
communication). Per core:
    fc1T = w1_e.T @ x_e.T        (f32r matmuls, PSUM fp32 accum over H)
    inter = silu(a) * b          (GLU on ACT+DVE straight out of PSUM)
    out_e = inter.T @ w2_e       (f32r, pair-wise PSUM accum + SBUF fp32 accumulator)

All matmuls run in float32r (TF32-like, 1 PE cycle/row at N=512 vs 4 for fp32;
measured ~1.5e-4 max rel err on K=2048 contractions).
"""

import sys

sys.path.insert(0, "/opt/trn_rl_repo")

import numpy as np

E, T, H, I = 8, 1024, 2048, 5632
TWO_I = 2 * I
P = 128
KO = H // P        # 16  k-subtiles for GEMM1
NJ = I // P        # 44  column tiles of I (= k-tiles for GEMM2)
NT = T // P        # 8   token tiles
NH = H // 512      # 4   output column tiles
N_CORES = 8

_RUNNER = None


def _build_program(reps: int = 1):
    import concourse.bacc as bacc
    import concourse.mybir as mybir
    import concourse.tile as tile
    from concourse.masks import make_identity

    f32 = mybir.dt.float32
    f32r = mybir.dt.float32r

    nc = bacc.Bacc("TRN2", target_bir_lowering=False, debug=False,
                   num_devices=N_CORES)
    x = nc.dram_tensor("x", [T, H], f32, kind="ExternalInput").ap()
    w1 = nc.dram_tensor("w1", [H, TWO_I], f32, kind="ExternalInput").ap()
    w2 = nc.dram_tensor("w2", [I, H], f32, kind="ExternalInput").ap()
    out = nc.dram_tensor("out", [T, H], f32, kind="ExternalOutput").ap()

    # K-on-partitions views (partition index is the inner row index)
    w1r = w1.bitcast(f32r).rearrange("(ko p) m -> p ko m", p=P)      # [128,16,11264]
    w2r = w2.bitcast(f32r).rearrange("(j p) h -> p j h", p=P)        # [128,44,2048]
    x_t = x.rearrange("(to p) h -> p to h", p=P)                     # [128,8,2048]
    out_t = out.rearrange("(to p) h -> p to h", p=P)                 # [128,8,2048]

    with tile.TileContext(nc) as tc:
        with (
            tc.tile_pool(name="const", bufs=1) as const,
            tc.tile_pool(name="xT", bufs=1) as xT_pool,
            tc.tile_pool(name="w1p", bufs=4) as w1p,
            tc.tile_pool(name="w2p", bufs=2) as w2p,
            tc.tile_pool(name="interp", bufs=3) as interp,
            tc.tile_pool(name="oacc", bufs=1) as oacc_pool,
        ):
            ident = const.tile([P, P], f32)
            make_identity(nc, ident)

            xT = xT_pool.tile([P, KO, T], f32r)            # 64 KB/partition
            out_acc = oacc_pool.tile([P, NT, H], f32)      # 64 KB/partition

            for _rep in range(reps):
                # ---- Phase A: transpose x into xT (stage via out_acc rows) ----
                with tc.tile_pool(name="ptr", bufs=4, space="PSUM") as ptr:
                    for to in range(NT):
                        stage = out_acc[:, to]             # [128, 2048] f32, dead space
                        nc.sync.dma_start(stage, x_t[:, to])
                        for ko in range(KO):
                            pst = ptr.tile([P, P], f32, tag="tr")
                            nc.tensor.transpose(
                                pst[:], stage[:, ko * P:(ko + 1) * P], ident[:]
                            )
                            nc.any.tensor_copy(
                                xT[:, ko, to * P:(to + 1) * P], pst[:]
                            )

                # ---- Main loop over I-tile pairs ----
                with (
                    tc.tile_pool(name="psum1", bufs=6, space="PSUM") as psum1,
                    tc.tile_pool(name="psum2", bufs=2, space="PSUM") as psum2,
                ):
                    inter_tiles = [None, None]
                    w2_tiles = [None, None]
                    for jp in range(NJ // 2):
                        for jj in range(2):
                            j = 2 * jp + jj
                            wa = w1p.tile([P, KO, P], f32r, tag="w1t")
                            wb = w1p.tile([P, KO, P], f32r, tag="w1t")
                            nc.sync.dma_start(wa[:], w1r[:, :, j * P:(j + 1) * P])
                            nc.sync.dma_start(
                                wb[:], w1r[:, :, (NJ + j) * P:(NJ + j + 1) * P]
                            )
                            it = interp.tile([P, T], f32r, tag="it")
                            for th in range(2):
                                pa = psum1.tile([P, 512], f32, tag="pg1")
                                pb = psum1.tile([P, 512], f32, tag="pg1")
                                for ko in range(KO):
                                    nc.tensor.matmul(
                                        pa[:], wa[:, ko],
                                        xT[:, ko, th * 512:(th + 1) * 512],
                                        start=(ko == 0), stop=(ko == KO - 1),
                                    )
                                for ko in range(KO):
                                    nc.tensor.matmul(
                                        pb[:], wb[:, ko],
                                        xT[:, ko, th * 512:(th + 1) * 512],
                                        start=(ko == 0), stop=(ko == KO - 1),
                                    )
                                # GLU: it = silu(a) * b  (silu on ACT, mul on DVE)
                                sl = it[:, th * 512:(th + 1) * 512]
                                nc.scalar.activation(
                                    sl, pa[:], mybir.ActivationFunctionType.Silu
                                )
                                nc.vector.tensor_tensor(
                                    sl, sl, pb[:], mybir.AluOpType.mult
                                )
                            inter_tiles[jj] = it
                            w2t = w2p.tile([P, H], f32r, tag="w2t")
                            nc.sync.dma_start(w2t[:], w2r[:, j])
                            w2_tiles[jj] = w2t

                        # GEMM2 partial for this pair, accumulate into out_acc
                        for t in range(NT):
                            for h in range(NH):
                                po = psum2.tile([P, 512], f32, tag="po")
                                nc.tensor.matmul(
                                    po[:],
                                    inter_tiles[0][:, t * P:(t + 1) * P],
                                    w2_tiles[0][:, h * 512:(h + 1) * 512],
                                    start=True, stop=False,
                                )
                                nc.tensor.matmul(
                                    po[:],
                                    inter_tiles[1][:, t * P:(t + 1) * P],
                                    w2_tiles[1][:, h * 512:(h + 1) * 512],
                                    start=False, stop=True,
                                )
                                dst = out_acc[:, t, h * 512:(h + 1) * 512]
                                if jp == 0:
                                    nc.vector.tensor_copy(dst, po[:])
                                else:
                                    nc.vector.tensor_tensor(
                                        dst, po[:], dst, mybir.AluOpType.add
                                    )

                # ---- Output ----
                nc.sync.dma_start(out_t[:], out_acc[:])

    nc.compile()
    return nc


def _build_runner(nc):
    import jax
    from jax.sharding import Mesh, PartitionSpec
    from jax.experimental.shard_map import shard_map
    import concourse.mybir as mybir
    from concourse.bass2jax import (
        _bass_exec_p, install_neuronx_cc_hook, partition_id_tensor,
    )

    install_neuronx_cc_hook()
    partition_name = (
        nc.partition_id_tensor.name if nc.partition_id_tensor else None
    )
    in_names, out_names, out_avals, zero_shapes = [], [], [], []
    for alloc in nc.m.functions[0].allocations:
        if not isinstance(alloc, mybir.MemoryLocationSet):
            continue
        name = alloc.memorylocations[0].name
        if alloc.kind == "ExternalInput":
            if name != partition_name:
                in_names.append(name)
        elif alloc.kind == "ExternalOutput":
            out_names.append(name)
            shape = tuple(alloc.tensor_shape)
            dtype = mybir.dt.np(alloc.dtype)
            out_avals.append(jax.core.ShapedArray(shape, dtype))
            zero_shapes.append((shape, dtype))
    n_params = len(in_names)
    n_outs = len(out_avals)
    all_in_names = list(in_names) + list(out_names)
    if partition_name is not None:
        all_in_names.append(partition_name)

    def _body(*args):
        operands = list(args)
        if partition_name is not None:
            operands.append(partition_id_tensor())
        outs = _bass_exec_p.bind(
            *operands,
            out_avals=tuple(out_avals),
            in_names=tuple(all_in_names),
            out_names=tuple(out_names),
            lowering_input_output_aliases=(),
            sim_require_finite=True,
            sim_require_nnan=True,
            nc=nc,
        )
        return tuple(outs)

    devices = jax.devices()[:N_CORES]
    mesh = Mesh(np.asarray(devices), ("core",))
    in_specs = (PartitionSpec("core"),) * (n_params + n_outs)
    out_specs = (PartitionSpec("core"),) * n_outs
    sharded = jax.jit(
        shard_map(_body, mesh=mesh, in_specs=in_specs, out_specs=out_specs,
                  check_rep=False),
        keep_unused=True,
    )

    def run(in_maps):
        concat_in = [
            np.concatenate([np.asarray(m[n]) for m in in_maps], axis=0)
            for n in in_names
        ]
        concat_zeros = [
            np.zeros((N_CORES * s[0], *s[1:]), dt) for s, dt in zero_shapes
        ]
        out_arrs = sharded(*concat_in, *concat_zeros)
        return [
            {n: np.asarray(out_arrs[i]).reshape(N_CORES, *out_avals[i].shape)[c]
             for i, n in enumerate(out_names)}
            for c in range(N_CORES)
        ]

    run.sharded = sharded
    run.in_names = in_names
    run.zero_shapes = zero_shapes
    return run


def _get_runner(reps: int = 1):
    global _RUNNER
    if _RUNNER is None or _RUNNER[1] != reps:
        nc = _build_program(reps)
        _RUNNER = (_build_runner(nc), reps)
    return _RUNNER[0]


def kernel(permuted_hidden_states, w1, w2, tokens_per_expert):
    run = _get_runner()
    phs = np.ascontiguousarray(np.asarray(permuted_hidden_states, dtype=np.float32))
    w1 = np.asarray(w1, dtype=np.float32)
    w2 = np.asarray(w2, dtype=np.float32)
    in_maps = [
        {
            "x": phs[e * T:(e + 1) * T],
            "w1": np.ascontiguousarray(w1[e]),
            "w2": np.ascontiguousarray(w2[e]),
        }
        for e in range(E)
    ]
    res = run(in_maps)
    return np.concatenate([res[e]["out"] for e in range(E)], axis=0)
